# revision 9
# baseline (speedup 1.0000x reference)
"""Multi-head attention (16 heads, D=128) on 8 trn2 NeuronCores.

Sharding: tensor-parallel over heads — each core owns 2 heads.
Per core: qkv projection for its 768 channels (chan-major for q/k,
token-major for v), fused RMSNorm+RoPE on q/k, SDPA in transposed-score
layout, partial proj over its 256 channels.  Host sums the 8 partial
outputs + bias (v-bias is folded into the host-side bias:
sum_k a*(v+b) = AV + b*d, so after the 1/d normalize it is a constant
per-channel offset that flows linearly through proj).

Matmul operands are fp16 (FWL weight load, full PE rate); accumulation
fp32 in PSUM.  exp(s/sqrt(D) - 4) keeps fp16 exp values in range.

Softmax denominator: DVE accumulates the exp tiles (fp16), GPSIMD
partition_all_reduce sums across partitions (replicated), DVE
reciprocal_approx_fast gives 1/d — no PE ones-matmuls, no ACT ln/exp.

RMSNorm rows: the 4 sum-of-squares matmuls of a token group land on
partitions 0/32/64/96 of one PSUM tile (column tiling), so one
partition-strided Ln + Exp pair serves all 4 (vs 8 row ops).

Layouts (per core):
  xT       [C=2048, TOK=4096]  (x transposed on host; tokens = b*2048+n)
  w_qk     SBUF [128, 16, 512]  lhsT tiles; chan-tiles = [q_h0,q_h1,k_h0,k_h1]
  w_v      SBUF [128, 16, 256]  rhs tiles (token-major v production)
  qT/kT    SBUF [128, 2, 2048]  D-major per head, per batch
  v        SBUF [128, 16, 256]  token-major per batch
  exp      SBUF [128, 16, 512]  exp(scores^T) per 512-wide q-chunk
  wpT      SBUF [128, 2, 2048]  proj rhs tiles
  y        DRAM [4096, 2048]    fp16 partial output (host adds cores + bias)
"""
import math
from contextlib import ExitStack

import numpy as np

import concourse.bass as bass
import concourse.bass_isa as bass_isa
import concourse.mybir as mybir
import concourse.tile as tile
from concourse import bacc, bass_utils

F32 = mybir.dt.float32
F16 = mybir.dt.float16

H, D, B, N, C = 16, 128, 2, 2048, 2048
NCORES = 8
HPC = H // NCORES            # heads per core = 2
TOK = B * N                  # 4096
EPS = float(np.finfo(np.float32).eps)
SCALE = 1.0 / math.sqrt(D)
ESHIFT = -4.0                # exp(s*SCALE + ESHIFT); softmax-invariant

_CACHE = {}
RUN_KW = {}   # test.py sets {"trace": True}


def _pin_act_table():
    """Restrict Exp/Ln to the combined natural_log_exp_and_others set so the
    table-load pass keeps a single ACT table resident."""
    import concourse.hw_specs as hw
    tabs = hw.get_activation_tables("gen3")
    for name, funcs in tabs.items():
        if name != "natural_log_exp_and_others":
            funcs.discard(mybir.ActivationFunctionType.Exp)
            funcs.discard(mybir.ActivationFunctionType.Ln)


def build_module():
    """Build + compile the per-core Bass module (same NEFF for all cores)."""
    if "nc" in _CACHE:
        return _CACHE["nc"]
    _pin_act_table()
    nc = bacc.Bacc("TRN2", target_bir_lowering=False, debug=False,
                   num_devices=NCORES)

    xt_h = nc.dram_tensor("xt", [C, TOK], F16, kind="ExternalInput")
    wqk_h = nc.dram_tensor("wqk", [C, 4 * 128], F16, kind="ExternalInput")
    wv_h = nc.dram_tensor("wv", [C, 2 * 128], F16, kind="ExternalInput")
    wp_h = nc.dram_tensor("wp", [2 * 128, C], F16, kind="ExternalInput")
    cos2_h = nc.dram_tensor("cos2", [128, N], F16, kind="ExternalInput")
    sin2_h = nc.dram_tensor("sin2", [128, N], F16, kind="ExternalInput")
    qkb_h = nc.dram_tensor("qkb", [128, 4], F32, kind="ExternalInput")
    invg2_h = nc.dram_tensor("invg2", [128, 64], F16, kind="ExternalInput")
    oner_h = nc.dram_tensor("oner", [128, 128], F16, kind="ExternalInput")
    eps_h = nc.dram_tensor("eps", [128, 1], F32, kind="ExternalInput")
    nb4_h = nc.dram_tensor("nb4", [128, 1], F32, kind="ExternalInput")
    y_h = nc.dram_tensor("y", [TOK, C], F16, kind="ExternalOutput")

    with tile.TileContext(nc) as tc, ExitStack() as ctx:
        pc = ctx.enter_context(tc.tile_pool(name="consts", bufs=1))
        p_xt = ctx.enter_context(tc.tile_pool(name="xt", bufs=3))
        p_qkv = ctx.enter_context(tc.tile_pool(name="qkv", bufs=1))
        p_qraw = ctx.enter_context(tc.tile_pool(name="qraw", bufs=2))
        p_qsw = ctx.enter_context(tc.tile_pool(name="qsw", bufs=3))
        p_sq = ctx.enter_context(tc.tile_pool(name="sq", bufs=3))
        p_row = ctx.enter_context(tc.tile_pool(name="rows", bufs=4))
        p_exp = ctx.enter_context(tc.tile_pool(name="exp", bufs=2))
        p_acc = ctx.enter_context(tc.tile_pool(name="acc", bufs=4))
        p_attn = ctx.enter_context(tc.tile_pool(name="attn", bufs=4))
        p_ao = ctx.enter_context(tc.tile_pool(name="ao", bufs=1))
        p_y = ctx.enter_context(tc.tile_pool(name="y", bufs=8))

        # weights needed first (first chunk's matmuls)
        wqk = pc.tile([128, 16, 512], F16)
        wv = pc.tile([128, 16, 256], F16)
        for hf in range(2):
            sl = slice(hf * 1024, (hf + 1) * 1024)
            nc.sync.dma_start(wqk[:, hf * 8:(hf + 1) * 8, :],
                              wqk_h.ap()[sl].rearrange("(t p) j -> p t j", p=128))
            nc.sync.dma_start(wv[:, hf * 8:(hf + 1) * 8, :],
                              wv_h.ap()[sl].rearrange("(t p) j -> p t j", p=128))
        qkb = pc.tile([128, 4], F32)
        nc.sync.dma_start(qkb[:], qkb_h.ap())
        invg2 = pc.tile([128, 2, 32], F16)
        nc.sync.dma_start(invg2[:], invg2_h.ap().rearrange("p (g r) -> p g r", g=2))
        oner = pc.tile([128, 128], F16)
        nc.sync.dma_start(oner[:], oner_h.ap())
        eps_t = pc.tile([128, 1], F32)
        nc.sync.dma_start(eps_t[:], eps_h.ap())
        nb4 = pc.tile([128, 1], F32)
        nc.sync.dma_start(nb4[:], nb4_h.ap())
        # deferred constants (not needed until first ph2 / proj)
        cos2 = pc.tile([128, N], F16)
        sin2 = pc.tile([128, N], F16)
        wp = pc.tile([128, 2, 2048], F16)
        deferred = {"rope": False, "wp": False}

        def load_rope_consts():
            if not deferred["rope"]:
                nc.sync.dma_start(cos2[:], cos2_h.ap())
                nc.sync.dma_start(sin2[:], sin2_h.ap())
                deferred["rope"] = True

        def load_wp():
            if not deferred["wp"]:
                nc.sync.dma_start(wp[:],
                                  wp_h.ap().rearrange("(t p) j -> p t j", p=128))
                deferred["wp"] = True

        for b in range(B):
            # ======== stage A: qkv projection for batch b ========
            pso = {}
            psA = tc.alloc_tile_pool(name=f"psA{b}", bufs=8, space="PSUM")
            pso["A"] = psA
            qT = p_qkv.tile([128, HPC, N], F16, tag="qT")
            kT = p_qkv.tile([128, HPC, N], F16, tag="kT")
            vtok = p_qkv.tile([128, 16, 256], F16, tag="v")
            qraw = None
            ph1_pend = []   # deferred sq/ss/ln/exp of the previous group
            ph2_pend = []   # deferred rs-broadcast + rope of the previous group

            def ph1(qraw_g, g0, gi):
                # squares (DVE) then 4 column-tiled sum-of-square matmuls
                # landing on partitions 0/32/64/96 of one PSUM tile, so a
                # single partition-strided Ln+Exp pair serves all 4 rows.
                sqs = []
                for ct in range(4):
                    src_q = qraw_g[:, ct, :]
                    sq = p_sq.tile([128, 512], F16, tag="sq", name=f"sq{b}{gi}{ct}")
                    nc.vector.tensor_mul(out=sq[:], in0=src_q, in1=src_q)
                    sqs.append(sq)
                ss4 = pso["A"].tile([128, 512], F32, tag="ss", bufs=1, name=f"ss{b}{gi}")
                for ct in range(4):
                    is_k = ct // 2
                    nc.tensor.matmul(ss4[32 * ct:32 * ct + 32, :],
                                     invg2[:, is_k, :], sqs[ct][:],
                                     start=True, stop=True,
                                     tile_position=(0, 32 * ct))
                l4 = p_row.tile([128, 512], F32, tag="l4", name=f"l4{b}{gi}")
                nc.scalar.activation(l4[:], ss4[:],
                                     mybir.ActivationFunctionType.Ln,
                                     bias=eps_t[:], scale=1.0 / D)
                r4 = p_row.tile([128, 512], F16, tag="r4", name=f"r4{b}{gi}")
                nc.scalar.activation(r4[:], l4[:],
                                     mybir.ActivationFunctionType.Exp,
                                     scale=-0.5)
                return r4

            def ph2(qraw_g, g0, gi, r4):
                load_rope_consts()
                for ct in range(4):
                    hl, is_k = ct % 2, ct // 2
                    dst = (kT if is_k else qT)
                    src_q = qraw_g[:, ct, :]
                    ps_rs = pso["A"].tile([128, 512], F32, tag="rs", bufs=1, name=f"rs{b}{gi}{ct}")
                    nc.tensor.matmul(ps_rs[:], oner[32 * ct:32 * ct + 1, :],
                                     r4[32 * ct:32 * ct + 1, :],
                                     start=True, stop=True,
                                     tile_position=(32 * ct, 0))
                    qsw = p_qsw.tile([128, 512], F16, tag="qsw", name=f"qsw{b}{gi}{ct}")
                    nc.sync.dma_start(qsw[0:64, :], src_q[64:128, :])
                    nc.sync.dma_start(qsw[64:128, :], src_q[0:64, :])
                    # in-place: qc into qraw, qs into qsw
                    nc.vector.tensor_mul(out=src_q, in0=src_q,
                                         in1=cos2[:, g0:g0 + 512])
                    nc.vector.tensor_mul(out=qsw[:], in0=qsw[:],
                                         in1=sin2[:, g0:g0 + 512])
                    rot = dst[:, hl, g0:g0 + 512]
                    nc.vector.tensor_add(out=rot, in0=src_q, in1=qsw[:])
                    nc.vector.tensor_mul(out=rot, in0=rot, in1=ps_rs[:])

            for ch in range(8):           # 256-token chunks
                tok0 = b * N + ch * 256
                if ch % 2 == 0:
                    qraw = p_qraw.tile([128, 4, 512], F16)
                off = (ch % 2) * 256
                ps_qk = [psA.tile([128, 256], F32, tag="ps", bufs=6,
                                  name=f"a{b}{ch}{ct}")[:] for ct in range(4)]
                ps_v = [psA.tile([128, 256], F32, tag="ps", bufs=6,
                                 name=f"av{b}{ch}{s}")[:] for s in range(2)]
                for half in range(2):
                    xt = p_xt.tile([128, 8, 256], F16)
                    src = xt_h.ap()[half * 1024:(half + 1) * 1024,
                                    tok0:tok0 + 256]
                    nc.sync.dma_start(xt[:], src.rearrange("(t p) j -> p t j", p=128))
                    for ct in range(4):
                        for kt in range(8):
                            nc.tensor.matmul(
                                ps_qk[ct], wqk[:, half * 8 + kt, ct * 128:(ct + 1) * 128],
                                xt[:, kt, :],
                                start=(half == 0 and kt == 0), stop=(half == 1 and kt == 7))
                    for s in range(2):
                        for kt in range(8):
                            nc.tensor.matmul(
                                ps_v[s], xt[:, kt, s * 128:(s + 1) * 128],
                                wv[:, half * 8 + kt, :],
                                start=(half == 0 and kt == 0), stop=(half == 1 and kt == 7))
                    # inject deferred epilogues mid-stream so their PE/ACT
                    # latency hides behind this chunk's dense matmuls
                    if half == 0 and ph1_pend:
                        args = ph1_pend.pop()
                        ph2_pend.append((args[0], args[1], args[2], ph1(*args)))
                    elif half == 1 and ph2_pend:
                        ph2(*ph2_pend.pop())
                for ct in range(4):
                    nc.vector.tensor_scalar_add(qraw[:, ct, off:off + 256],
                                                ps_qk[ct], qkb[:, ct:ct + 1])
                for s in range(2):
                    nc.scalar.copy(vtok[:, ch * 2 + s, :], ps_v[s])
                if ch % 2 == 1:
                    ph1_pend.append((qraw, (ch - 1) * 256, ch // 2))
            # flush the last group's epilogue
            args = ph1_pend.pop()
            ph2(args[0], args[1], args[2], ph1(*args))
            if b == 0:
                load_wp()
            psA.release()

            # ======== SDPA for (b, h0) and (b, h1) ========
            psB = tc.alloc_tile_pool(name=f"psB{b}", bufs=3, space="PSUM")
            ao = p_ao.tile([128, 2, N], F16)   # attn out^T, stays in SBUF
            pend = []                          # deferred normalize tails

            def normalize(hl, qc, accA, ps_av):
                q0 = qc * 512
                d32 = p_attn.tile([128, 512], F32, tag="d32", name=f"d{b}{hl}{qc}")
                nc.gpsimd.partition_all_reduce(d32[:], accA[:], 128,
                                               bass_isa.ReduceOp.add)
                rd = p_attn.tile([128, 512], F32, tag="rd", name=f"rd{b}{hl}{qc}")
                nc.vector.reciprocal_approx_fast(out=rd[:], in_=d32[:])
                nc.vector.tensor_mul(out=ao[:, hl, q0:q0 + 512],
                                     in0=ps_av[:], in1=rd[:])

            for hl in range(HPC):
                for qc in range(4):       # 512-wide q chunks
                    q0 = qc * 512
                    ex = p_exp.tile([128, 16, 512], F16)
                    accA = p_acc.tile([128, 512], F16, tag="A", name=f"aA{b}{hl}{qc}")
                    accB = p_acc.tile([128, 512], F16, tag="B", name=f"aB{b}{hl}{qc}")
                    ps_s = [None] * 8
                    ps_av = None

                    def qkg(g):
                        # one score group: 2 QK matmuls into a 2-bank tile
                        ps_s[g] = psB.tile([128, 2, 512], F32, tag="s", bufs=3,
                                           name=f"s{b}{hl}{qc}{g}")
                        for j in range(2):
                            kt = 2 * g + j
                            nc.tensor.matmul(ps_s[g][:, j, :],
                                             kT[:, hl, kt * 128:(kt + 1) * 128],
                                             qT[:, hl, q0:q0 + 512],
                                             start=True, stop=True)

                    def tailg(g):
                        # exp of group g (one [128,1024] ACT op), AV matmuls,
                        # and fp16 accumulation for the softmax denominator
                        nc.scalar.activation(ex[:, 2 * g:2 * g + 2, :], ps_s[g][:],
                                             mybir.ActivationFunctionType.Exp,
                                             bias=nb4[:], scale=SCALE)
                        for j in range(2):
                            kt = 2 * g + j
                            nc.tensor.matmul(ps_av[:],
                                             vtok[:, kt, hl * 128:(hl + 1) * 128],
                                             ex[:, kt, :],
                                             start=(kt == 0), stop=(kt == 15))
                        if g == 0:
                            nc.vector.tensor_add(out=accA[:], in0=ex[:, 0, :],
                                                 in1=ex[:, 1, :])
                        elif g == 1:
                            nc.vector.tensor_add(out=accB[:], in0=ex[:, 2, :],
                                                 in1=ex[:, 3, :])
                        else:
                            nc.vector.tensor_add(out=accA[:], in0=accA[:],
                                                 in1=ex[:, 2 * g, :])
                            nc.vector.tensor_add(out=accB[:], in0=accB[:],
                                                 in1=ex[:, 2 * g + 1, :])

                    for g in range(8):
                        qkg(g)
                        if g == 1:
                            if pend:
                                normalize(*pend.pop())
                            ps_av = psB.tile([128, 512], F32, tag="av", bufs=2,
                                             name=f"o{b}{hl}{qc}")
                        if g >= 1:
                            tailg(g - 1)
                    tailg(7)
                    nc.vector.tensor_add(out=accA[:], in0=accA[:], in1=accB[:])
                    pend.append((hl, qc, accA, ps_av))
            normalize(*pend.pop())
            psB.release()

            # ======== stage C: partial proj for batch b ========
            psC = tc.alloc_tile_pool(name=f"psC{b}", bufs=6, space="PSUM")
            for tt in range(16):          # 128-token tiles
                for oc in range(4):
                    ps_y = psC.tile([128, 512], F32, tag="y", name=f"y{b}{tt}{oc}")
                    for ct in range(2):
                        nc.tensor.matmul(ps_y[:], ao[:, ct, tt * 128:(tt + 1) * 128],
                                         wp[:, ct, oc * 512:(oc + 1) * 512],
                                         start=(ct == 0), stop=(ct == 1))
                    yt = p_y.tile([128, 512], F16)
                    if oc % 2 == 0:
                        nc.vector.tensor_copy(yt[:], ps_y[:])
                    else:
                        nc.scalar.copy(yt[:], ps_y[:])
                    nc.sync.dma_start(
                        y_h.ap()[b * N + tt * 128:b * N + (tt + 1) * 128,
                                 oc * 512:(oc + 1) * 512], yt[:])
            psC.release()

    nc.compile()
    _CACHE["nc"] = nc
    return nc


def make_in_maps(x, rope, qkv_w, qkv_b, proj_w, q_norm_w, k_norm_w):
    """Host-side prep: transpose x, slice/scale weights per core."""
    x = np.asarray(x, np.float32)
    rope = np.asarray(rope, np.float32)
    qkv_w = np.asarray(qkv_w, np.float32)
    qkv_b = np.asarray(qkv_b, np.float32)
    proj_w = np.asarray(proj_w, np.float32)
    g_q = np.asarray(q_norm_w, np.float32)
    g_k = np.asarray(k_norm_w, np.float32)
    if np.any(g_q == 0) or np.any(g_k == 0):
        raise ValueError("zero rmsnorm weight not supported")

    xt = np.ascontiguousarray(x.reshape(TOK, C).T.astype(np.float16))  # [C, TOK]
    cos = np.cos(rope)                                        # [N, 64]
    sin = np.sin(rope)
    cos2 = np.ascontiguousarray(
        np.concatenate([cos, cos], axis=1).T.astype(np.float16))       # [128, N]
    sin2 = np.ascontiguousarray(
        np.concatenate([-sin, sin], axis=1).T.astype(np.float16))      # [128, N]
    invg2 = np.repeat(
        np.stack([1.0 / g_q ** 2, 1.0 / g_k ** 2], axis=1), 32, axis=1
    ).astype(np.float16)                                               # [128, 64]
    oner = np.ones((128, 128), np.float16)
    eps = np.full((128, 1), EPS, np.float32)
    nb4 = np.full((128, 1), ESHIFT, np.float32)

    in_maps = []
    for c in range(NCORES):
        hs = [HPC * c + hl for hl in range(HPC)]
        # chan-tiles: q_h0, q_h1, k_h0, k_h1 (g-scaled rows + bias)
        rows, biases = [], []
        for base, g in ((0, g_q), (C, g_k)):
            for h in hs:
                r0 = base + h * D
                rows.append(qkv_w[r0:r0 + D] * g[:, None])
                biases.append(qkv_b[r0:r0 + D] * g)
        wqk = np.ascontiguousarray(
            np.concatenate(rows, axis=0).T.astype(np.float16))           # [C, 512]
        qkb = np.stack(biases, axis=1)                                   # [128, 4]
        vrows = [qkv_w[2 * C + h * D:2 * C + (h + 1) * D] for h in hs]
        wv = np.ascontiguousarray(
            np.concatenate(vrows, axis=0).T.astype(np.float16))          # [C, 256]
        cols = np.concatenate([np.arange(h * D, (h + 1) * D) for h in hs])
        wpT = np.ascontiguousarray(proj_w[:, cols].T.astype(np.float16))  # [256, C]
        in_maps.append({
            "xt": xt, "wqk": wqk, "wv": wv, "wp": wpT,
            "cos2": cos2, "sin2": sin2, "qkb": qkb,
            "invg2": invg2, "oner": oner,
            "eps": eps, "nb4": nb4,
        })
    return in_maps


def kernel(x, rope, qkv_w, qkv_b, proj_w, proj_b, q_norm_w, k_norm_w):
    nc = build_module()
    in_maps = make_in_maps(x, rope, qkv_w, qkv_b, proj_w, q_norm_w, k_norm_w)
    res = bass_utils.run_bass_kernel_spmd(nc, in_maps,
                                          core_ids=list(range(NCORES)), **RUN_KW)
    _CACHE["last_result"] = res
    y = np.zeros((TOK, C), np.float64)
    for c in range(NCORES):
        y += res.results[c]["y"].astype(np.float64)
    # host bias: proj_b plus the folded-out v-bias contribution
    vb = np.asarray(qkv_b, np.float64)[2 * C:3 * C]
    y += np.asarray(proj_b, np.float64) + np.asarray(proj_w, np.float64) @ vb
    return y.astype(np.float32).reshape(B, N, C)


# revision 13
# speedup vs baseline: 1.1300x; 1.1300x over previous
"""Multi-head attention (16 heads, D=128) on 8 trn2 NeuronCores.

Sharding: tensor-parallel over heads — each core owns 2 heads.
Per core: qkv projection for its 768 channels (chan-major for q/k,
token-major for v), fused RMSNorm+RoPE on q/k, SDPA in transposed-score
layout, partial proj over its 256 channels.  Host sums the 8 partial
outputs + bias (v-bias is folded into the host-side bias:
sum_k a*(v+b) = AV + b*d, so after the 1/d normalize it is a constant
per-channel offset that flows linearly through proj).

Cross-phase software pipeline (keeps the PE fed while ACT does exp):
  P0: qkv(batch0)
  P1: SDPA(batch0) interleaved with qkv(batch1)
  P2: SDPA(batch1) interleaved with proj(batch0) + rolling proj(batch1)
  P3: tail of proj(batch1)

Matmul operands are fp16 (FWL weight load, full PE rate); accumulation
fp32 in PSUM.  exp(s/sqrt(D) - 4) keeps fp16 exp values in range.

Softmax denominator: DVE accumulates the exp tiles (fp16), GPSIMD
partition_all_reduce sums across partitions (replicated), DVE
reciprocal_approx_fast gives 1/d — no PE ones-matmuls, no ACT ln/exp.

RMSNorm rows: the 4 sum-of-squares matmuls of a token group land on
partition blocks 0/32/64/96 of one PSUM tile (column tiling, rows
replicated 32x via a widened inverse-gain lhs), so one full-tile Ln +
Exp pair serves all 4 groups.

PSUM (8 banks, one static layout):  "s" score tiles [128,2,512] x2
(4 banks) + "av" [128,512] x2 (2 banks) + "m" ring x2 (2 banks, shared
by qkv accumulators / rmsnorm rows / proj outputs).
"""
import math
from contextlib import ExitStack

import numpy as np

import concourse.bass as bass
import concourse.bass_isa as bass_isa
import concourse.mybir as mybir
import concourse.tile as tile
from concourse import bacc, bass_utils

F32 = mybir.dt.float32
F16 = mybir.dt.float16

H, D, B, N, C = 16, 128, 2, 2048, 2048
NCORES = 8
HPC = H // NCORES            # heads per core = 2
TOK = B * N                  # 4096
EPS = float(np.finfo(np.float32).eps)
SCALE = 1.0 / math.sqrt(D)
ESHIFT = -4.0                # exp(s*SCALE + ESHIFT); softmax-invariant

_CACHE = {}
RUN_KW = {}   # test.py sets {"trace": True}


def _pin_act_table():
    """Restrict Exp/Ln to the combined natural_log_exp_and_others set so the
    table-load pass keeps a single ACT table resident."""
    import concourse.hw_specs as hw
    tabs = hw.get_activation_tables("gen3")
    for name, funcs in tabs.items():
        if name != "natural_log_exp_and_others":
            funcs.discard(mybir.ActivationFunctionType.Exp)
            funcs.discard(mybir.ActivationFunctionType.Ln)


def build_module():
    """Build + compile the per-core Bass module (same NEFF for all cores)."""
    if "nc" in _CACHE:
        return _CACHE["nc"]
    _pin_act_table()
    nc = bacc.Bacc("TRN2", target_bir_lowering=False, debug=False,
                   num_devices=NCORES)

    xt_h = nc.dram_tensor("xt", [C, TOK], F16, kind="ExternalInput")
    wqk_h = nc.dram_tensor("wqk", [C, 4 * 128], F16, kind="ExternalInput")
    wv_h = nc.dram_tensor("wv", [C, 2 * 128], F16, kind="ExternalInput")
    wp_h = nc.dram_tensor("wp", [2 * 128, C], F16, kind="ExternalInput")
    cos2_h = nc.dram_tensor("cos2", [128, N], F16, kind="ExternalInput")
    sin2_h = nc.dram_tensor("sin2", [128, N], F16, kind="ExternalInput")
    qkb_h = nc.dram_tensor("qkb", [128, 4], F32, kind="ExternalInput")
    invg2_h = nc.dram_tensor("invg2", [128, 64], F16, kind="ExternalInput")
    oner_h = nc.dram_tensor("oner", [128, 128], F16, kind="ExternalInput")
    eps_h = nc.dram_tensor("eps", [128, 1], F32, kind="ExternalInput")
    nb4_h = nc.dram_tensor("nb4", [128, 1], F32, kind="ExternalInput")
    y_h = nc.dram_tensor("y", [TOK, C], F16, kind="ExternalOutput")

    with tile.TileContext(nc) as tc, ExitStack() as ctx:
        pc = ctx.enter_context(tc.tile_pool(name="consts", bufs=1))
        p_xt = ctx.enter_context(tc.tile_pool(name="xt", bufs=4))
        p_qkv = ctx.enter_context(tc.tile_pool(name="qkv", bufs=2))
        p_qraw = ctx.enter_context(tc.tile_pool(name="qraw", bufs=2))
        p_qsw = ctx.enter_context(tc.tile_pool(name="qsw", bufs=3))
        p_sq = ctx.enter_context(tc.tile_pool(name="sq", bufs=3))
        p_row = ctx.enter_context(tc.tile_pool(name="rows", bufs=2))
        p_exp = ctx.enter_context(tc.tile_pool(name="exp", bufs=2))
        p_acc = ctx.enter_context(tc.tile_pool(name="acc", bufs=2))
        p_attn = ctx.enter_context(tc.tile_pool(name="attn", bufs=2))
        p_ao = ctx.enter_context(tc.tile_pool(name="ao", bufs=2))
        p_y = ctx.enter_context(tc.tile_pool(name="y", bufs=8))
        p_s = ctx.enter_context(tc.tile_pool(name="pss", bufs=2, space="PSUM"))
        p_av = ctx.enter_context(tc.tile_pool(name="psav", bufs=2, space="PSUM"))
        p_m = ctx.enter_context(tc.tile_pool(name="psm", bufs=2, space="PSUM"))

        # weights needed first (first chunk's matmuls)
        wqk = pc.tile([128, 16, 512], F16)
        wv = pc.tile([128, 16, 256], F16)
        for hf in range(2):
            sl = slice(hf * 1024, (hf + 1) * 1024)
            nc.sync.dma_start(wqk[:, hf * 8:(hf + 1) * 8, :],
                              wqk_h.ap()[sl].rearrange("(t p) j -> p t j", p=128))
            nc.sync.dma_start(wv[:, hf * 8:(hf + 1) * 8, :],
                              wv_h.ap()[sl].rearrange("(t p) j -> p t j", p=128))
        qkb = pc.tile([128, 4], F32)
        nc.sync.dma_start(qkb[:], qkb_h.ap())
        invg2 = pc.tile([128, 2, 32], F16)
        nc.sync.dma_start(invg2[:], invg2_h.ap().rearrange("p (g r) -> p g r", g=2))
        oner = pc.tile([128, 128], F16)
        nc.sync.dma_start(oner[:], oner_h.ap())
        eps_t = pc.tile([128, 1], F32)
        nc.sync.dma_start(eps_t[:], eps_h.ap())
        nb4 = pc.tile([128, 1], F32)
        nc.sync.dma_start(nb4[:], nb4_h.ap())
        # deferred constants (not needed until first ph2 / proj)
        cos2 = pc.tile([128, N], F16)
        sin2 = pc.tile([128, N], F16)
        wp = pc.tile([128, 2, 2048], F16)
        deferred = {"rope": False, "wp": False}

        def load_rope_consts():
            if not deferred["rope"]:
                nc.sync.dma_start(cos2[:], cos2_h.ap())
                nc.sync.dma_start(sin2[:], sin2_h.ap())
                deferred["rope"] = True

        def load_wp():
            if not deferred["wp"]:
                nc.sync.dma_start(wp[:],
                                  wp_h.ap().rearrange("(t p) j -> p t j", p=128))
                deferred["wp"] = True

        dat = [{} for _ in range(B)]   # per-batch qT/kT/vtok/ao tiles

        def ph1(b, qraw_g, g0, gi):
            """RMSNorm row stats for one 512-token group: squares (DVE),
            4 column-tiled sum-of-square matmuls (rows replicated 32x so
            the whole [128,512] tile is defined), one Ln + Exp pair."""
            sqs = []
            for ct in range(4):
                src_q = qraw_g[:, ct, :]
                sq = p_sq.tile([128, 512], F16, tag="sq", name=f"sq{b}{gi}{ct}")
                nc.vector.tensor_mul(out=sq[:], in0=src_q, in1=src_q)
                sqs.append(sq)
            ss4 = p_m.tile([128, 512], F32, tag="m", name=f"ss{b}{gi}")
            for ct in range(4):
                is_k = ct // 2
                nc.tensor.matmul(ss4[32 * ct:32 * ct + 32, :],
                                 invg2[:, is_k, :], sqs[ct][:],
                                 start=True, stop=True,
                                 tile_position=(0, 32 * ct))
            l4 = p_row.tile([128, 512], F32, tag="l4", name=f"l4{b}{gi}")
            nc.scalar.activation(l4[:], ss4[:],
                                 mybir.ActivationFunctionType.Ln,
                                 bias=eps_t[:], scale=1.0 / D)
            r4 = p_row.tile([128, 512], F16, tag="r4", name=f"r4{b}{gi}")
            nc.scalar.activation(r4[:], l4[:],
                                 mybir.ActivationFunctionType.Exp,
                                 scale=-0.5)
            return r4

        def ph2_ct(b, qraw_g, g0, gi, r4, ct):
            """RoPE + 1/rms scale for one chan-tile of a token group."""
            qT, kT = dat[b]["qT"], dat[b]["kT"]
            hl, is_k = ct % 2, ct // 2
            dst = (kT if is_k else qT)
            src_q = qraw_g[:, ct, :]
            ps_rs = p_m.tile([128, 512], F32, tag="m", name=f"rs{b}{gi}{ct}")
            nc.tensor.matmul(ps_rs[:], oner[32 * ct:32 * ct + 1, :],
                             r4[32 * ct:32 * ct + 1, :],
                             start=True, stop=True,
                             tile_position=(32 * ct, 0))
            qsw = p_qsw.tile([128, 512], F16, tag="qsw", name=f"qsw{b}{gi}{ct}")
            nc.sync.dma_start(qsw[0:64, :], src_q[64:128, :])
            nc.sync.dma_start(qsw[64:128, :], src_q[0:64, :])
            # in-place: qc into qraw, qs into qsw
            nc.vector.tensor_mul(out=src_q, in0=src_q, in1=cos2[:, g0:g0 + 512])
            nc.vector.tensor_mul(out=qsw[:], in0=qsw[:], in1=sin2[:, g0:g0 + 512])
            rot = dst[:, hl, g0:g0 + 512]
            nc.vector.tensor_add(out=rot, in0=src_q, in1=qsw[:])
            nc.vector.tensor_mul(out=rot, in0=rot, in1=ps_rs[:])

        def qkv_gen(b, wide):
            """Emit the qkv projection for batch b, yielding between ~1us
            emission units so a driver can interleave other work.

            wide=True (P0, rings free): all 6 chunk accumulators live
            (m x2 + av x2 + s x2).  wide=False (P1): only the m ring is
            used (2 banks) — chan-tiles produced in waves of two.
            """
            qT = p_qkv.tile([128, HPC, N], F16, tag="qT", name=f"qT{b}")
            kT = p_qkv.tile([128, HPC, N], F16, tag="kT", name=f"kT{b}")
            vtok = p_qkv.tile([128, 16, 256], F16, tag="v", name=f"vt{b}")
            dat[b].update(qT=qT, kT=kT, vtok=vtok)
            qraw = None
            ph_pend = []

            for ch in range(8):           # 256-token chunks
                tok0 = b * N + ch * 256
                if ch % 2 == 0:
                    qraw = p_qraw.tile([128, 4, 512], F16, tag="qraw", name=f"qraw{b}{ch}")
                off = (ch % 2) * 256
                xts = []
                for half in range(2):
                    xt = p_xt.tile([128, 8, 256], F16, tag="xt", name=f"x{b}{ch}{half}")
                    src = xt_h.ap()[half * 1024:(half + 1) * 1024,
                                    tok0:tok0 + 256]
                    nc.sync.dma_start(xt[:], src.rearrange("(t p) j -> p t j", p=128))
                    xts.append(xt)

                def qk_mms(ct, acc):
                    for half in range(2):
                        for kt in range(8):
                            nc.tensor.matmul(
                                acc[:], wqk[:, half * 8 + kt, ct * 128:(ct + 1) * 128],
                                xts[half][:, kt, :],
                                start=(half == 0 and kt == 0),
                                stop=(half == 1 and kt == 7))

                def v_mms(s, acc):
                    for half in range(2):
                        for kt in range(8):
                            nc.tensor.matmul(
                                acc[:], xts[half][:, kt, s * 128:(s + 1) * 128],
                                wv[:, half * 8 + kt, :],
                                start=(half == 0 and kt == 0),
                                stop=(half == 1 and kt == 7))

                def qk_drain(ct, acc):
                    nc.vector.tensor_scalar_add(qraw[:, ct, off:off + 256],
                                                acc[:], qkb[:, ct:ct + 1])

                def v_drain(s, acc):
                    nc.scalar.copy(vtok[:, ch * 2 + s, :], acc[:])

                if wide:
                    accs = [p_m.tile([128, 256], F32, tag="m", name=f"a{b}{ch}0"),
                            p_m.tile([128, 256], F32, tag="m", name=f"a{b}{ch}1"),
                            p_av.tile([128, 256], F32, tag="av", name=f"a{b}{ch}2"),
                            p_av.tile([128, 256], F32, tag="av", name=f"a{b}{ch}3")]
                    vaccs = [p_s.tile([128, 256], F32, tag="s", name=f"av{b}{ch}{s}")
                             for s in range(2)]
                    # half-major streaming like the baseline
                    for half in range(2):
                        for ct in range(4):
                            for kt in range(8):
                                nc.tensor.matmul(
                                    accs[ct][:],
                                    wqk[:, half * 8 + kt, ct * 128:(ct + 1) * 128],
                                    xts[half][:, kt, :],
                                    start=(half == 0 and kt == 0),
                                    stop=(half == 1 and kt == 7))
                        for s in range(2):
                            for kt in range(8):
                                nc.tensor.matmul(
                                    vaccs[s][:], xts[half][:, kt, s * 128:(s + 1) * 128],
                                    wv[:, half * 8 + kt, :],
                                    start=(half == 0 and kt == 0),
                                    stop=(half == 1 and kt == 7))
                        yield
                    for ct in range(4):
                        qk_drain(ct, accs[ct])
                    for s in range(2):
                        v_drain(s, vaccs[s])
                    yield
                else:
                    for wave in ((0, 1), (2, 3)):
                        waccs = [p_m.tile([128, 256], F32, tag="m",
                                          name=f"a{b}{ch}{ct}") for ct in wave]
                        for i, ct in enumerate(wave):
                            qk_mms(ct, waccs[i])
                            yield
                        for i, ct in enumerate(wave):
                            qk_drain(ct, waccs[i])
                    vaccs = [p_m.tile([128, 256], F32, tag="m",
                                      name=f"av{b}{ch}{s}") for s in range(2)]
                    for s in range(2):
                        v_mms(s, vaccs[s])
                        yield
                    for s in range(2):
                        v_drain(s, vaccs[s])

                # rmsnorm + rope epilogue for each completed 512-token group
                if ch % 2 == 1:
                    ph_pend.append((qraw, (ch - 1) * 256, ch // 2))
                if ph_pend and (ch % 2 == 0 or ch == 7):
                    g_args = ph_pend.pop(0)
                    load_rope_consts()
                    r4 = ph1(b, *g_args)
                    yield
                    for ct in range(4):
                        ph2_ct(b, *g_args, r4, ct)
                        if ct == 1:
                            yield
                    yield

        def sdpa_pairs(b):
            """Pair order (qc, hl) so proj token-tiles unlock progressively."""
            for qc in range(4):
                for hl in range(HPC):
                    yield hl, qc

        pend = []   # deferred normalize across pairs AND phases

        def normalize(b, hl, qc, accA, ps_av):
            q0 = qc * 512
            ao = dat[b]["ao"]
            d32 = p_attn.tile([128, 512], F32, tag="d32", name=f"d{b}{hl}{qc}")
            nc.gpsimd.partition_all_reduce(d32[:], accA[:], 128,
                                           bass_isa.ReduceOp.add)
            rd = p_attn.tile([128, 512], F32, tag="rd", name=f"rd{b}{hl}{qc}")
            nc.vector.reciprocal_approx_fast(out=rd[:], in_=d32[:])
            nc.vector.tensor_mul(out=ao[:, hl, q0:q0 + 512],
                                 in0=ps_av[:], in1=rd[:])

        def sdpa_pair(b, hl, qc, filler):
            """One (head, q-chunk) SDPA pair; filler() is called between
            pipeline steps to slip in ~1us of independent PE work."""
            qT, kT, vtok = dat[b]["qT"], dat[b]["kT"], dat[b]["vtok"]
            q0 = qc * 512
            ex = p_exp.tile([128, 16, 512], F16, tag="ex", name=f"ex{b}{hl}{qc}")
            accA = p_acc.tile([128, 512], F16, tag="A", name=f"aA{b}{hl}{qc}")
            accB = p_acc.tile([128, 512], F16, tag="B", name=f"aB{b}{hl}{qc}")
            ps_s = [None] * 8
            ps_av = None

            def qkg(g):
                ps_s[g] = p_s.tile([128, 2, 512], F32, tag="s",
                                   name=f"s{b}{hl}{qc}{g}")
                for j in range(2):
                    kt = 2 * g + j
                    nc.tensor.matmul(ps_s[g][:, j, :],
                                     kT[:, hl, kt * 128:(kt + 1) * 128],
                                     qT[:, hl, q0:q0 + 512],
                                     start=True, stop=True)

            def tailg(g):
                nc.scalar.activation(ex[:, 2 * g:2 * g + 2, :], ps_s[g][:],
                                     mybir.ActivationFunctionType.Exp,
                                     bias=nb4[:], scale=SCALE)
                for j in range(2):
                    kt = 2 * g + j
                    nc.tensor.matmul(ps_av[:],
                                     vtok[:, kt, hl * 128:(hl + 1) * 128],
                                     ex[:, kt, :],
                                     start=(kt == 0), stop=(kt == 15))
                if g == 0:
                    nc.vector.tensor_add(out=accA[:], in0=ex[:, 0, :],
                                         in1=ex[:, 1, :])
                elif g == 1:
                    nc.vector.tensor_add(out=accB[:], in0=ex[:, 2, :],
                                         in1=ex[:, 3, :])
                else:
                    nc.vector.tensor_add(out=accA[:], in0=accA[:],
                                         in1=ex[:, 2 * g, :])
                    nc.vector.tensor_add(out=accB[:], in0=accB[:],
                                         in1=ex[:, 2 * g + 1, :])

            for g in range(8):
                qkg(g)
                if g == 1:
                    if pend:
                        normalize(*pend.pop(0))
                    ps_av = p_av.tile([128, 512], F32, tag="av",
                                      name=f"o{b}{hl}{qc}")
                if g >= 1:
                    tailg(g - 1)
                filler()
            tailg(7)
            nc.vector.tensor_add(out=accA[:], in0=accA[:], in1=accB[:])
            pend.append((b, hl, qc, accA, ps_av))

        def proj_tile(b, tt, oc2):
            """Two output chunks of one 128-token tile: 4 matmuls into a
            2-bank PSUM tile, one cast copy, one DMA."""
            ao = dat[b]["ao"]
            ps_y = p_s.tile([128, 2, 512], F32, tag="s", name=f"y{b}{tt}{oc2}")
            for j in range(2):
                oc = oc2 * 2 + j
                for ct in range(2):
                    nc.tensor.matmul(ps_y[:, j, :],
                                     ao[:, ct, tt * 128:(tt + 1) * 128],
                                     wp[:, ct, oc * 512:(oc + 1) * 512],
                                     start=(ct == 0), stop=(ct == 1))
            yt = p_y.tile([128, 1024], F16, tag="yt", name=f"yt{b}{tt}{oc2}")
            if (tt + oc2) % 2 == 0:
                nc.vector.tensor_copy(yt[:], ps_y[:])
            else:
                nc.scalar.copy(yt[:], ps_y[:])
            nc.sync.dma_start(
                y_h.ap()[b * N + tt * 128:b * N + (tt + 1) * 128,
                         oc2 * 1024:(oc2 + 1) * 1024], yt[:])

        # ---- P0: qkv(b0) ----
        for _ in qkv_gen(0, wide=True):
            pass

        # ---- P1: SDPA(b0) + qkv(b1) ----
        dat[0]["ao"] = p_ao.tile([128, 2, N], F16, tag="ao", name="ao0")
        g1 = qkv_gen(1, wide=False)
        g1_done = [False]

        def fill_from_g1():
            if not g1_done[0]:
                try:
                    next(g1)
                except StopIteration:
                    g1_done[0] = True

        npairs = 0
        for hl, qc in sdpa_pairs(0):
            npairs += 1
            if npairs == 6:
                load_wp()
            sdpa_pair(0, hl, qc, fill_from_g1)
        while not g1_done[0]:
            fill_from_g1()

        # ---- P2: SDPA(b1) + proj(b0) + rolling proj(b1) ----
        dat[1]["ao"] = p_ao.tile([128, 2, N], F16, tag="ao", name="ao1")
        proj_q = [(0, tt, oc2) for tt in range(16) for oc2 in range(2)]

        def fill_from_proj():
            if proj_q:
                proj_tile(*proj_q.pop(0))

        prev_pair = [None]
        for hl, qc in sdpa_pairs(1):
            sdpa_pair(1, hl, qc, fill_from_proj)
            # the previous pair's normalize was just emitted inside this
            # pair; once a q-chunk's hl=1 normalize is in, its proj token
            # tiles are unlockable
            if prev_pair[0] is not None and prev_pair[0][0] == 1:
                pqc = prev_pair[0][1]
                proj_q.extend((1, tt, oc2)
                              for tt in range(4 * pqc, 4 * pqc + 4)
                              for oc2 in range(2))
            prev_pair[0] = (hl, qc)

        # ---- P3: flush the last normalize and remaining proj tiles ----
        normalize(*pend.pop(0))
        proj_q.extend((1, tt, oc2) for tt in range(12, 16) for oc2 in range(2))
        while proj_q:
            proj_tile(*proj_q.pop(0))

    nc.compile()
    _CACHE["nc"] = nc
    return nc


def make_in_maps(x, rope, qkv_w, qkv_b, proj_w, q_norm_w, k_norm_w):
    """Host-side prep: transpose x, slice/scale weights per core."""
    x = np.asarray(x, np.float32)
    rope = np.asarray(rope, np.float32)
    qkv_w = np.asarray(qkv_w, np.float32)
    qkv_b = np.asarray(qkv_b, np.float32)
    proj_w = np.asarray(proj_w, np.float32)
    g_q = np.asarray(q_norm_w, np.float32)
    g_k = np.asarray(k_norm_w, np.float32)
    if np.any(g_q == 0) or np.any(g_k == 0):
        raise ValueError("zero rmsnorm weight not supported")

    xt = np.ascontiguousarray(x.reshape(TOK, C).T.astype(np.float16))  # [C, TOK]
    cos = np.cos(rope)                                        # [N, 64]
    sin = np.sin(rope)
    cos2 = np.ascontiguousarray(
        np.concatenate([cos, cos], axis=1).T.astype(np.float16))       # [128, N]
    sin2 = np.ascontiguousarray(
        np.concatenate([-sin, sin], axis=1).T.astype(np.float16))      # [128, N]
    invg2 = np.repeat(
        np.stack([1.0 / g_q ** 2, 1.0 / g_k ** 2], axis=1), 32, axis=1
    ).astype(np.float16)                                               # [128, 64]
    oner = np.ones((128, 128), np.float16)
    eps = np.full((128, 1), EPS, np.float32)
    nb4 = np.full((128, 1), ESHIFT, np.float32)

    in_maps = []
    for c in range(NCORES):
        hs = [HPC * c + hl for hl in range(HPC)]
        # chan-tiles: q_h0, q_h1, k_h0, k_h1 (g-scaled rows + bias)
        rows, biases = [], []
        for base, g in ((0, g_q), (C, g_k)):
            for h in hs:
                r0 = base + h * D
                rows.append(qkv_w[r0:r0 + D] * g[:, None])
                biases.append(qkv_b[r0:r0 + D] * g)
        wqk = np.ascontiguousarray(
            np.concatenate(rows, axis=0).T.astype(np.float16))           # [C, 512]
        qkb = np.stack(biases, axis=1)                                   # [128, 4]
        vrows = [qkv_w[2 * C + h * D:2 * C + (h + 1) * D] for h in hs]
        wv = np.ascontiguousarray(
            np.concatenate(vrows, axis=0).T.astype(np.float16))          # [C, 256]
        cols = np.concatenate([np.arange(h * D, (h + 1) * D) for h in hs])
        wpT = np.ascontiguousarray(proj_w[:, cols].T.astype(np.float16))  # [256, C]
        in_maps.append({
            "xt": xt, "wqk": wqk, "wv": wv, "wp": wpT,
            "cos2": cos2, "sin2": sin2, "qkb": qkb,
            "invg2": invg2, "oner": oner,
            "eps": eps, "nb4": nb4,
        })
    return in_maps


def kernel(x, rope, qkv_w, qkv_b, proj_w, proj_b, q_norm_w, k_norm_w):
    nc = build_module()
    in_maps = make_in_maps(x, rope, qkv_w, qkv_b, proj_w, q_norm_w, k_norm_w)
    res = bass_utils.run_bass_kernel_spmd(nc, in_maps,
                                          core_ids=list(range(NCORES)), **RUN_KW)
    _CACHE["last_result"] = res
    y = np.zeros((TOK, C), np.float64)
    for c in range(NCORES):
        y += res.results[c]["y"].astype(np.float64)
    # host bias: proj_b plus the folded-out v-bias contribution
    vb = np.asarray(qkv_b, np.float64)[2 * C:3 * C]
    y += np.asarray(proj_b, np.float64) + np.asarray(proj_w, np.float64) @ vb
    return y.astype(np.float32).reshape(B, N, C)
